# revision 14
# baseline (speedup 1.0000x reference)
"""DeepWelchTransform kernel for Trainium2 (8 NeuronCores).

Math
----
The reference computes, per batch row b (B=1024, S=16384, NPERSEG=1024,
STEP=256, NWIN=61):

    fr[b] = mean_w  sum_t input[b, 256*w + t] *  cos(2*pi*freqs[t])
    fi[b] = mean_w  sum_t input[b, 256*w + t] * (-sin(2*pi*freqs[t]))
    out[b] = (fr[b]^2 + fi[b]^2) * fc_w + fc_b

Everything up to the square is linear in `input`, so the window
gather + per-window dot + mean folds into a single length-S dot product
per batch row with "effective" weight vectors

    c_eff[s] = (1/61) * sum_{w : 0 <= s-256w < 1024} cos(ang[s-256w])
    s_eff[s] = (1/61) * sum_{w : 0 <= s-256w < 1024} -sin(ang[s-256w])

(the host folds these from `freqs` in float64 — O(S) work). The device
work is then two matvecs [1024, 16384] @ [16384] → purely HBM-bound
(64 MiB input read; ~23 us/core at the ~358 GB/s per-core HBM limit).

Sharding
--------
The sequence dim is split across the 8 cores (2048 s-positions each);
every core sees all 1024 batch rows and produces partial (fr, fi) pairs
for all rows. No on-device communication: the host sums the 8 partials
and applies the final square + affine (a few KFLOP on [1024]).

Per-core device kernel
----------------------
The 2048 s-positions map to 128 SBUF partitions x 16 columns
(s_local = p*16 + rs). For each rs, TensorE matmuls contract over the
128 partitions: stationary = [128, M] weight slices, moving = [128, 512]
batch slabs, accumulated over all 16 rs into PSUM. The input shard is
pre-arranged on the host to [p][rs][b] so the DMA is perfectly
sequential (64 KiB/partition).

Precision ("bf8p" default): x is split hi/lo as bf16 + scaled-fp8
residual (3 B/elem of DMA — 25% below the fp32 roofline's traffic).
The bf16 stationary packs (wh_c, wh_s, wl_c, wl_s) into M=4 columns so
one xh pass yields main + w-correction products simultaneously; the fp8
residual stream multiplies a scaled fp8 stationary (M=4 with a
second-order w correction) into its own PSUM group. The host unscales
and sums the PSUM rows. Measured max relative error vs the fp32
reference: ~2.8e-4 (scale-relative absmax ~3.5e-5). The "bf16p" variant
(4 B/elem, ~2.3e-5 max rel err) is one flag away.

PE warm-up junk matmuls run during the first chunk's DMA so the HAM
clock gate releases before real work; a small final DMA chunk keeps the
post-DMA matmul tail short. Modeled single-shot: ~25 us/core; steady
state is HBM-bound at ~6 MiB / core read.
"""

import numpy as np

import concourse.bass as bass
import concourse.tile as tile
from concourse import bacc, mybir
from concourse.bass_utils import run_bass_kernel_spmd

N_CORES = 8
B, S = 1024, 16384
NPERSEG, STEP = 1024, 256
NWIN = (S - NPERSEG) // STEP + 1  # 61
S_PC = S // N_CORES  # 2048 s-positions per core
P = 128  # SBUF partitions
RS = S_PC // P  # 16 s-columns per partition
N_HALF = 512  # moving free size (1024 batch cols / 2)
RS_PER_CHUNK = 2  # DMA chunk granularity (2 rs cols: 0.5 MiB per hi/lo DMA)
# full-size chunks + single-rs tail chunks (shorter post-DMA matmul tail)
N_CHUNKS = RS // RS_PER_CHUNK - 1 + RS_PER_CHUNK

_f32 = mybir.dt.float32
_f32r = mybir.dt.float32r
_bf16 = mybir.dt.bfloat16
_f16 = mybir.dt.float16
_f8 = mybir.dt.float8e4

# The fp8 residual stream (bf8p) pre-scales xl / wb / wb2 on the host with
# adaptive power-of-2 factors (chosen per call from the data's max-abs so
# e4m3 never saturates); the host divides the stream-B partials back down.

_NC_CACHE = {}


# Per-precision stream configs.
#   x: list of (name, dtype) moving tensors
#   w: list of (name, dtype, m) stationary tensors (m = packed column count)
#   streams: (x_idx, w_idx, group) matmul products; each group accumulates
#            into its own PSUM bank pair and emits its own [m, B] output.
_CONFIGS = {
    # bf16 hi/lo split of x AND w, with the w hi/lo (x cos/sin) packed into
    # the stationary's M columns: stationary [128, 4] = (whc, whs, wlc, wls).
    # Two moving passes (xh, xl) against the same stationary accumulate the
    # full (xh+xl)*(wh+wl) split across PSUM rows {0,2} (cos) / {1,3} (sin);
    # the host sums even/odd rows.  PE cost: 2 passes, M is free.
    "bf16p": {
        "x": [("xh", _bf16), ("xl", _bf16)],
        "w": [("w4", _bf16, 4)],
        "streams": [(0, 0, 0), (1, 0, 0)],
    },
    # like bf16p but the xl residual is shipped as scaled fp8 (3 B/elem of
    # DMA instead of 4) and multiplies a scaled fp8 copy of w in its own
    # PSUM group. The fp8 stationary carries 4 columns: (wb_c, wb_s) plus a
    # second-order correction pair (wb2_c, wb2_s) encoding the fp8
    # quantization error of wb — M-packing makes the correction free. The
    # host unscales group-1 rows {0,1} by 1/(XL_SCALE*WB_SCALE) and rows
    # {2,3} by 1/(XL_SCALE*WB2_SCALE).
    "bf8p": {
        "x": [("xh", _bf16), ("xl", _f8)],
        "w": [("w4", _bf16, 4), ("wb", _f8, 4)],
        "streams": [(0, 0, 0), (1, 1, 1)],
    },
    # single fp16 stream: x shipped as fp16 (2 B/elem — 50% of the fp32
    # roofline's traffic, 2/3 of bf8p's), stationary fp16 [128, 4] packing
    # (whc, whs, wlc*2^10, wls*2^10) — the w residual columns ride free in M.
    # The host divides PSUM rows {2,3} by 2^10 and sums hi+lo pairs. One
    # moving pass also halves the PE work. Measured max rel err ~1.5e-3.
    "f16p": None,  # dedicated builder (_build_bass_f16)
    "f32r": {
        "x": [("x", _f32r)],
        "w": [("w", _f32r, 2)],
        "streams": [(0, 0, 0)],
    },
    "f16x2": {
        "x": [("xh", _f16), ("xl", _f16)],
        "w": [("wh", _f16, 2)],
        "streams": [(0, 0, 0), (1, 0, 0)],
    },
    "bf16x3": {
        "x": [("xh", _bf16), ("xl", _bf16)],
        "w": [("wh", _bf16, 2), ("wl", _bf16, 2)],
        "streams": [(0, 0, 0), (0, 1, 0), (1, 0, 0)],
    },
}


WL_SCALE = 1024.0  # keeps the fp16 w-residual columns in normal range

# x chunk column ranges (per-core x is [P, RS*B] fp16; columns are rs-major,
# batch-minor). Full 2-rs chunks stream at peak DMA rate. The tail is
# reordered so bank 1 (batch cols 512:1024) finishes first: rs15's h1 half
# ships before h0, and h0 arrives as two 256-col quarters so its final
# matmuls + PSUM drains start as early as possible.
_Q = N_HALF // 2  # 256
F16_CHUNKS = [(rs * B, 2 * B) for rs in range(0, 14, 2)] + [
    (14 * B, B),  # rs14
    (15 * B + N_HALF, N_HALF),  # rs15 h1
    (15 * B, _Q),  # rs15 h0 q0
    (15 * B + _Q, _Q),  # rs15 h0 q1
]
SCATTER_NIDX = 16  # one 16-partition wrap row; entries 4..15 are -1 (ignored)
# The prepared-scatter writeback (saves the out-DMA's HWDGE+DGE issue latency,
# ~1.2 us) runs in CoreSim but the axon fake-NRT terminal dies executing the
# custom InstDMAScatterAddAnt ucode, so it's disabled for the graded path.
USE_SCATTER_OUT = False


def _build_bass_f16(repeat=1):
    assert sum(n for _, n in F16_CHUNKS) == RS * B
    nc = bacc.Bacc("TRN2", debug=False)
    o_d = nc.dram_tensor("o", [4, B], _f32, kind="ExternalOutput").ap()
    x_d = nc.dram_tensor("xh", [P, RS * B], _f16, kind="ExternalInput").ap()
    w_d = nc.dram_tensor("w4", [P, RS * 4], _f16, kind="ExternalInput").ap()
    idx_d = nc.dram_tensor("sidx", [P, 1], mybir.dt.int16, kind="ExternalInput").ap()

    with tile.TileContext(nc) as tc:
        with (
            tc.tile_pool(name="xp", bufs=len(F16_CHUNKS)) as xp,
            tc.tile_pool(name="wp", bufs=1) as wp,
            tc.tile_pool(name="pp", bufs=1, space="PSUM") as pp,
            tc.tile_pool(name="op", bufs=1) as op,
        ):
            # weights + scatter idxs via SWDGE (gpsimd) so the x chunk DMAs
            # own the HWDGE ring from t=0
            w_sb = wp.tile([P, RS * 4], _f16, name="w_sb", tag="w")
            nc.gpsimd.dma_start(w_sb[:], w_d[:])
            idx_sb = wp.tile([P, 1], mybir.dt.int16, name="idx_sb", tag="idx")
            nc.gpsimd.dma_start(idx_sb[:], idx_d[:])

            # result staging for the scatter writeback: rows 0..3 hold the
            # PSUM drains; the rest must be initialized (the interp reads the
            # whole AP) but is never scattered (idxs 4.. are -1)
            osrc = op.tile([P, B], _f32, name="osrc", tag="osrc")
            nc.vector.memset(osrc[:], 0.0)

            # PE warm-up junk matmuls: keep the PE busy during the first
            # chunk's DMA so the p-state ramp (0.65 -> 1.2 -> 2.4 GHz after
            # 3 us busy) completes before the real matmuls start. The last
            # one reads w_sb so the first real matmul carries a single
            # sync-wait.
            junk = wp.tile([P, N_HALF], _f16, name="junk", tag="junk")
            nc.vector.memset(junk[:], 0.0)
            scratch = pp.tile([4, N_HALF], _f32, name="scratch", tag="scratch", bufs=1)
            for _ in range(8):
                nc.tensor.matmul(
                    scratch[:2, :], junk[:, 0:2], junk[:], start=True, stop=True
                )
            nc.tensor.matmul(
                scratch[:4, 0 : RS * 4], w_sb[:, 0:4], w_sb[:], start=True, stop=True
            )

            for it in range(repeat):
                ps = {
                    h: pp.tile([4, N_HALF], _f32, name=f"ps{h}_{it}", tag=f"ps{h}")
                    for h in range(2)
                }
                use_scatter = USE_SCATTER_OUT
                if use_scatter:
                    dma_sem = nc.alloc_semaphore(f"osem_{it}")
                    # scatter writeback prepared up front (descriptor gen
                    # hidden under the x stream); the trigger at the end fires
                    # it with no HWDGE/DGE issue latency on the critical path.
                    # o[i] += row i of osrc for i in 0..3 (o is
                    # zero-initialized by the runtime).
                    nc.gpsimd.dma_scatter_add(
                        o_d[:],
                        osrc[:].unsqueeze(1),
                        idx_sb[:],
                        SCATTER_NIDX,
                        4,
                        B,
                        prepare_only=True,
                        sem=dma_sem,
                    )
                for c, (col0, ncols) in enumerate(F16_CHUNKS):
                    x_c = xp.tile([P, ncols], _f16, name=f"x_{it}_{c}", tag="x")
                    nc.sync.dma_start(x_c[:], x_d[:, col0 : col0 + ncols])
                    # matmul each 512/256-col piece into its bank (bank =
                    # batch-half); start on rs0, stop on rs15
                    off = 0
                    while off < ncols:
                        bcol = (col0 + off) % B  # batch col within the row
                        rs = (col0 + off) // B
                        h = bcol // N_HALF
                        n = min(N_HALF - bcol % N_HALF, ncols - off)
                        if h == 0 and rs > 0:
                            # bank 0 accumulates in 256-col pieces so the
                            # final quarters can stop (and drain) separately.
                            # rs0 must stay one 512-wide matmul: start=True
                            # zeroes the whole 2 KiB zero region, so zeroing
                            # must happen exactly once per bank.
                            n = min(n, _Q)
                        nc.tensor.matmul(
                            ps[h][:, bcol % N_HALF : bcol % N_HALF + n],
                            w_sb[:, 4 * rs : 4 * (rs + 1)],
                            x_c[:, off : off + n],
                            start=(rs == 0),
                            stop=(rs == RS - 1),
                            # bank 0's 256-col accumulate regions are
                            # sub-zero-region; the interp's group check is
                            # 2 KiB-granular and would reject them
                            skip_group_check=(h == 0),
                        )
                        if rs == RS - 1:
                            # drain each PSUM piece as soon as its
                            # accumulation stops: bank 1 on ACT under the h0
                            # quarters' DMAs/matmuls, h0 quarters on DVE
                            if h == 1:
                                nc.scalar.copy(
                                    osrc[:4, N_HALF:B], ps[1][:]
                                )
                            else:
                                nc.vector.tensor_copy(
                                    osrc[:4, bcol : bcol + n],
                                    ps[0][:, bcol : bcol + n],
                                )
                        off += n
                if use_scatter:
                    nc.gpsimd.trigger_dma(count=None)
                else:
                    nc.sync.dma_start(o_d[:], osrc[:4, :])
    nc.compile()
    return nc


def _build_bass(repeat=1, precision="bf8p"):
    if precision == "f16p":
        return _build_bass_f16(repeat)
    cfg = _CONFIGS[precision]
    x_specs, w_specs, streams = cfg["x"], cfg["w"], cfg["streams"]
    groups = sorted({g for _, _, g in streams})
    grp_m = {g: max(w_specs[wi][2] for _, wi, gg in streams if gg == g) for g in groups}
    m_max = max(grp_m.values())

    nc = bacc.Bacc("TRN2", debug=False)
    # one output tensor: group g occupies columns [g*B, (g+1)*B) — both
    # PSUM groups sit on partitions 0..m-1, so a single SBUF staging tile
    # and a single out-DMA cover all of them
    o_d = nc.dram_tensor(
        "o", [m_max, len(groups) * B], _f32, kind="ExternalOutput"
    ).ap()
    x_ds = [
        nc.dram_tensor(n, [P, RS * B], dt, kind="ExternalInput").ap()
        for n, dt in x_specs
    ]
    w_ds = [
        nc.dram_tensor(n, [P, RS * m], dt, kind="ExternalInput").ap()
        for n, dt, m in w_specs
    ]

    with tile.TileContext(nc) as tc:
        with (
            tc.tile_pool(name="xp", bufs=N_CHUNKS) as xp,
            tc.tile_pool(name="wp", bufs=1) as wp,
            tc.tile_pool(name="pp", bufs=1, space="PSUM") as pp,
            tc.tile_pool(name="op", bufs=2) as op,
        ):
            # weights go via SWDGE (gpsimd) so the x chunk DMAs own the
            # HWDGE ring from t=0
            w_sbs = []
            for i, (n, dt, m) in enumerate(w_specs):
                w_sb = wp.tile([P, RS * m], dt, name=f"w_sb{i}", tag=f"w{i}")
                nc.gpsimd.dma_start(w_sb[:], w_ds[i][:])
                w_sbs.append(w_sb)

            # PE warm-up: junk matmuls on a zeroed tile keep the PE busy
            # during the first chunk's DMA so the HAM clock-gate releases
            # (1.2 -> 2.4 GHz) before the real matmuls start. The final
            # junk matmuls read the w_sb tiles so the first real matmul
            # carries a single sync-wait (the fused LDW+MM pair has few
            # wait slots).
            junk = wp.tile([P, N_HALF], x_specs[0][1], name="junk", tag="junk")
            nc.vector.memset(junk[:], 0.0)
            scratch = pp.tile(
                [max(grp_m.values()), N_HALF],
                _f32,
                name="scratch",
                tag="scratch",
                bufs=1,
            )
            for _ in range(8):
                nc.tensor.matmul(
                    scratch[:2, :], junk[:, 0:2], junk[:], start=True, stop=True
                )
            for i, w_sb in enumerate(w_sbs):
                m = w_specs[i][2]
                nc.tensor.matmul(
                    scratch[:m, 0 : RS * m],
                    w_sb[:, 0:m],
                    w_sb[:],
                    start=True,
                    stop=True,
                )

            # chunk sizes in rs columns; a small final chunk shortens the
            # post-DMA matmul tail
            chunk_rs = [RS_PER_CHUNK] * (RS // RS_PER_CHUNK - 1) + [1] * (
                RS_PER_CHUNK
            )
            assert sum(chunk_rs) == RS

            first_s = {
                g: next(s for s in streams if s[2] == g) for g in groups
            }
            last_s = {
                g: next(s for s in reversed(streams) if s[2] == g)
                for g in groups
            }

            for it in range(repeat):
                ps = {
                    (g, h): pp.tile(
                        [grp_m[g], N_HALF],
                        _f32,
                        name=f"ps{g}_{h}_{it}",
                        tag=f"ps{g}_{h}",
                    )
                    for g in groups
                    for h in range(2)
                }
                rs0 = 0
                for c, crs in enumerate(chunk_rs):
                    x_cs = []
                    for i, (n, dt) in enumerate(x_specs):
                        x_c = xp.tile(
                            [P, crs * B], dt, name=f"x{i}_{it}_{c}", tag=f"x{i}"
                        )
                        nc.sync.dma_start(
                            x_c[:], x_ds[i][:, rs0 * B : (rs0 + crs) * B]
                        )
                        x_cs.append(x_c)
                    for r in range(crs):
                        rs = rs0 + r
                        last_rs = rs == RS - 1
                        # streams stay in xh-first order: on the final column
                        # the xh-dependent matmuls start as soon as xh lands
                        # (xl arrives last), and each bank's copy launches
                        # right after its own final matmul
                        s_order = streams
                        for s in s_order:
                            xi, wi, g = s
                            m = w_specs[wi][2]
                            lhsT = w_sbs[wi][:, m * rs : m * (rs + 1)]
                            # on the final column, finish bank 1 first so its
                            # PSUM->SBUF copy overlaps bank 0's last matmuls
                            h_order = (1, 0) if last_rs else (0, 1)
                            for h in h_order:
                                rhs = x_cs[xi][
                                    :, r * B + h * N_HALF : r * B + (h + 1) * N_HALF
                                ]
                                nc.tensor.matmul(
                                    ps[(g, h)][:],
                                    lhsT,
                                    rhs,
                                    start=(rs == 0 and s == first_s[g]),
                                    stop=(last_rs and s == last_s[g]),
                                )
                    rs0 += crs

                out_sb = op.tile(
                    [m_max, len(groups) * B],
                    _f32,
                    name=f"out_sb_{it}",
                    tag="out_sb",
                )
                for g in groups:
                    m = grp_m[g]
                    nc.vector.tensor_copy(
                        out_sb[:m, g * B + N_HALF : (g + 1) * B], ps[(g, 1)][:]
                    )
                    nc.scalar.copy(
                        out_sb[:m, g * B : g * B + N_HALF], ps[(g, 0)][:]
                    )
                nc.sync.dma_start(o_d[:], out_sb[:])
    nc.compile()
    return nc


def _get_nc(repeat=1, precision="bf8p"):
    key = (repeat, precision)
    if key not in _NC_CACHE:
        _NC_CACHE[key] = _build_bass(repeat, precision)
    return _NC_CACHE[key]


def _fold_weights(freqs):
    """Fold freqs -> effective per-position cos/sin weights [S, 2] (f32)."""
    ang = 2.0 * np.pi * np.asarray(freqs, dtype=np.float64)
    cosv = np.cos(ang)
    msinv = -np.sin(ang)
    c_eff = np.zeros(S, np.float64)
    s_eff = np.zeros(S, np.float64)
    for w in range(NWIN):
        c_eff[w * STEP : w * STEP + NPERSEG] += cosv
        s_eff[w * STEP : w * STEP + NPERSEG] += msinv
    c_eff /= NWIN
    s_eff /= NWIN
    return np.stack([c_eff, s_eff], axis=-1).astype(np.float32)  # [S, 2]


def _pow2_scale(max_abs, target=120.0):
    """Largest power-of-2 scale keeping max_abs*scale <= target.

    ml_dtypes.float8_e4m3 (IEEE, used for mybir float8e4) has max finite
    240 and overflows to inf — stay at half that."""
    if max_abs <= 0 or not np.isfinite(max_abs):
        return 1.0
    return float(2.0 ** np.floor(np.log2(target / max_abs)))


def _run_f16(input, freqs, fc_w, fc_b, trace=False):
    input = np.ascontiguousarray(np.asarray(input, dtype=np.float32))
    eff = _fold_weights(freqs)  # [S, 2] f32

    # device layout x[p, rs*B + b] = shard[b, p*RS + rs]
    x_dev = input.reshape(B, N_CORES, P, RS).transpose(1, 2, 3, 0)
    w_dev = eff.reshape(N_CORES, P, RS, 2)

    sidx = np.full((P, 1), -1, np.int16)
    sidx[:4, 0] = np.arange(4)

    in_maps = []
    for k in range(N_CORES):
        xh = np.ascontiguousarray(
            x_dev[k].reshape(P, RS * B).astype(np.float16)
        )
        w2 = w_dev[k].astype(np.float64)
        wh = w2.astype(np.float16)
        wl = ((w2 - wh.astype(np.float64)) * WL_SCALE).astype(np.float16)
        w4 = np.ascontiguousarray(
            np.concatenate([wh, wl], axis=-1).reshape(P, RS * 4)
        )
        in_maps.append({"xh": xh, "w4": w4, "sidx": sidx})

    last_exc = None
    for attempt in range(3):
        try:
            res = run_bass_kernel_spmd(
                _get_nc(1, "f16p"),
                in_maps,
                core_ids=list(range(N_CORES)),
                trace=trace,
            )
            break
        except Exception as e:  # transient NRT/device hiccups: retry
            last_exc = e
            import time as _time

            _time.sleep(2.0)
    else:
        raise last_exc

    fr = np.zeros(B, np.float64)
    fi = np.zeros(B, np.float64)
    for r in res.results:
        o = r["o"].astype(np.float64)  # [4, B]: (hi_c, hi_s, lo_c, lo_s)
        fr += o[0] + o[2] / WL_SCALE
        fi += o[1] + o[3] / WL_SCALE
    psd = fr**2 + fi**2
    out = psd * float(np.asarray(fc_w).reshape(-1)[0]) + float(
        np.asarray(fc_b).reshape(-1)[0]
    )
    return out.astype(np.float32).reshape(B, 1), res


def _run(input, freqs, fc_w, fc_b, trace=False, precision="bf8p"):
    if precision == "f16p":
        return _run_f16(input, freqs, fc_w, fc_b, trace=trace)
    input = np.ascontiguousarray(np.asarray(input, dtype=np.float32))
    eff = _fold_weights(freqs)

    # rearrange to the device layout x[p, rs*B + b] = shard[b, p*RS + rs]
    x_dev = np.ascontiguousarray(
        input.reshape(B, N_CORES, P, RS).transpose(1, 2, 3, 0)
    )  # [N_CORES, P, RS, B]
    w_dev = eff.reshape(N_CORES, P, RS * 2)

    # adaptive (host-side only) fp8 scales: the device multiplies scaled
    # values, the host divides the partials back down
    scales = {}
    if precision == "bf8p":
        import ml_dtypes

        f8_np = mybir.dt.np(_f8)
        xl_all = input - input.astype(ml_dtypes.bfloat16).astype(np.float32)
        scales["xl"] = _pow2_scale(np.abs(xl_all).max())
        scales["wb"] = _pow2_scale(np.abs(eff).max())
        werr_all = eff - (eff * scales["wb"]).astype(f8_np).astype(
            np.float32
        ) / scales["wb"]
        scales["wb2"] = _pow2_scale(np.abs(werr_all).max())
        del xl_all, werr_all

    in_maps = []
    for k in range(N_CORES):
        x_host = x_dev[k].reshape(P, RS * B)
        w_host = w_dev[k]
        if precision in ("bf16p", "bf8p"):
            import ml_dtypes

            xh = x_host.astype(ml_dtypes.bfloat16)
            xl_f32 = x_host - xh.astype(np.float32)
            w2 = w_host.reshape(P, RS, 2)
            wh = w2.astype(ml_dtypes.bfloat16)
            wl = (w2 - wh.astype(np.float32)).astype(ml_dtypes.bfloat16)
            w4 = np.concatenate([wh, wl], axis=-1).reshape(P, RS * 4)
            m = {"xh": xh, "w4": np.ascontiguousarray(w4)}
            if precision == "bf16p":
                m["xl"] = np.ascontiguousarray(xl_f32.astype(ml_dtypes.bfloat16))
            else:
                f8 = mybir.dt.np(_f8)
                xl_s, wb_s, wb2_s = scales["xl"], scales["wb"], scales["wb2"]
                m["xl"] = np.ascontiguousarray((xl_f32 * xl_s).astype(f8))
                wb = (w2 * wb_s).astype(f8)
                werr = w2 - wb.astype(np.float32) / wb_s
                wb2 = (werr * wb2_s).astype(f8)
                m["wb"] = np.ascontiguousarray(
                    np.concatenate([wb, wb2], axis=-1).reshape(P, RS * 4)
                )
            in_maps.append(m)
        elif precision == "f32r":
            in_maps.append({"x": x_host, "w": np.ascontiguousarray(w_host)})
        elif precision == "f16x2":
            xh = x_host.astype(np.float16)
            xl = (x_host - xh.astype(np.float32)).astype(np.float16)
            wh = np.ascontiguousarray(w_host).astype(np.float16)
            in_maps.append({"xh": xh, "xl": np.ascontiguousarray(xl), "wh": wh})
        else:
            import ml_dtypes

            xh = x_host.astype(ml_dtypes.bfloat16)
            xl = (x_host - xh.astype(np.float32)).astype(ml_dtypes.bfloat16)
            wh = w_host.astype(ml_dtypes.bfloat16)
            wl = (w_host - wh.astype(np.float32)).astype(ml_dtypes.bfloat16)
            in_maps.append(
                {
                    "xh": xh,
                    "xl": np.ascontiguousarray(xl),
                    "wh": np.ascontiguousarray(wh),
                    "wl": np.ascontiguousarray(wl),
                }
            )

    last_exc = None
    for attempt in range(3):
        try:
            res = run_bass_kernel_spmd(
                _get_nc(1, precision),
                in_maps,
                core_ids=list(range(N_CORES)),
                trace=trace,
            )
            break
        except Exception as e:  # transient NRT/device hiccups: retry
            last_exc = e
            import time as _time

            _time.sleep(2.0)
    else:
        raise last_exc

    fr = np.zeros(B, np.float64)
    fi = np.zeros(B, np.float64)
    for r in res.results:
        o = r["o"]
        g0 = o[:, 0:B]
        fr += g0[0::2].sum(axis=0, dtype=np.float64)
        fi += g0[1::2].sum(axis=0, dtype=np.float64)
        if o.shape[1] > B:  # fp8 residual group (bf8p)
            g1 = o[:, B : 2 * B]
            s1 = scales["xl"] * scales["wb"]
            s2 = scales["xl"] * scales["wb2"]
            fr += g1[0].astype(np.float64) / s1
            fi += g1[1].astype(np.float64) / s1
            fr += g1[2].astype(np.float64) / s2
            fi += g1[3].astype(np.float64) / s2
    psd = fr**2 + fi**2
    out = psd * float(np.asarray(fc_w).reshape(-1)[0]) + float(
        np.asarray(fc_b).reshape(-1)[0]
    )
    return out.astype(np.float32).reshape(B, 1), res


def kernel(input, freqs, fc_w, fc_b):
    out, _ = _run(input, freqs, fc_w, fc_b, trace=False, precision="f16p")
    return out



# revision 19
# speedup vs baseline: 1.0347x; 1.0347x over previous
"""DeepWelchTransform kernel for Trainium2 (8 NeuronCores).

Math
----
The reference computes, per batch row b (B=1024, S=16384, NPERSEG=1024,
STEP=256, NWIN=61):

    fr[b] = mean_w  sum_t input[b, 256*w + t] *  cos(2*pi*freqs[t])
    fi[b] = mean_w  sum_t input[b, 256*w + t] * (-sin(2*pi*freqs[t]))
    out[b] = (fr[b]^2 + fi[b]^2) * fc_w + fc_b

Everything up to the square is linear in `input`, so the window
gather + per-window dot + mean folds into a single length-S dot product
per batch row with "effective" weight vectors

    c_eff[s] = (1/61) * sum_{w : 0 <= s-256w < 1024} cos(ang[s-256w])
    s_eff[s] = (1/61) * sum_{w : 0 <= s-256w < 1024} -sin(ang[s-256w])

(the host folds these from `freqs` in float64 — O(S) work). The device
work is then two matvecs [1024, 16384] @ [16384] → purely HBM-bound
(64 MiB input read; ~23 us/core at the ~358 GB/s per-core HBM limit).

Sharding
--------
The sequence dim is split across the 8 cores (2048 s-positions each);
every core sees all 1024 batch rows and produces partial (fr, fi) pairs
for all rows. No on-device communication: the host sums the 8 partials
and applies the final square + affine (a few KFLOP on [1024]).

Per-core device kernel
----------------------
The 2048 s-positions map to 128 SBUF partitions x 16 columns
(s_local = p*16 + rs). For each rs, TensorE matmuls contract over the
128 partitions: stationary = [128, M] weight slices, moving = [128, 512]
batch slabs, accumulated over all 16 rs into PSUM. The input shard is
pre-arranged on the host to [p][rs][b] so the DMA is perfectly
sequential (64 KiB/partition).

Precision ("bf8p" default): x is split hi/lo as bf16 + scaled-fp8
residual (3 B/elem of DMA — 25% below the fp32 roofline's traffic).
The bf16 stationary packs (wh_c, wh_s, wl_c, wl_s) into M=4 columns so
one xh pass yields main + w-correction products simultaneously; the fp8
residual stream multiplies a scaled fp8 stationary (M=4 with a
second-order w correction) into its own PSUM group. The host unscales
and sums the PSUM rows. Measured max relative error vs the fp32
reference: ~2.8e-4 (scale-relative absmax ~3.5e-5). The "bf16p" variant
(4 B/elem, ~2.3e-5 max rel err) is one flag away.

PE warm-up junk matmuls run during the first chunk's DMA so the HAM
clock gate releases before real work; a small final DMA chunk keeps the
post-DMA matmul tail short. Modeled single-shot: ~25 us/core; steady
state is HBM-bound at ~6 MiB / core read.
"""

import numpy as np

import concourse.bass as bass
import concourse.tile as tile
from concourse import bacc, mybir
from concourse.bass_utils import run_bass_kernel_spmd

N_CORES = 8
B, S = 1024, 16384
NPERSEG, STEP = 1024, 256
NWIN = (S - NPERSEG) // STEP + 1  # 61
S_PC = S // N_CORES  # 2048 s-positions per core
P = 128  # SBUF partitions
RS = S_PC // P  # 16 s-columns per partition
N_HALF = 512  # moving free size (1024 batch cols / 2)
RS_PER_CHUNK = 2  # DMA chunk granularity (2 rs cols: 0.5 MiB per hi/lo DMA)
# full-size chunks + single-rs tail chunks (shorter post-DMA matmul tail)
N_CHUNKS = RS // RS_PER_CHUNK - 1 + RS_PER_CHUNK

_f32 = mybir.dt.float32
_f32r = mybir.dt.float32r
_bf16 = mybir.dt.bfloat16
_f16 = mybir.dt.float16
_f8 = mybir.dt.float8e4

# The fp8 residual stream (bf8p) pre-scales xl / wb / wb2 on the host with
# adaptive power-of-2 factors (chosen per call from the data's max-abs so
# e4m3 never saturates); the host divides the stream-B partials back down.

_NC_CACHE = {}


# Per-precision stream configs.
#   x: list of (name, dtype) moving tensors
#   w: list of (name, dtype, m) stationary tensors (m = packed column count)
#   streams: (x_idx, w_idx, group) matmul products; each group accumulates
#            into its own PSUM bank pair and emits its own [m, B] output.
_CONFIGS = {
    # bf16 hi/lo split of x AND w, with the w hi/lo (x cos/sin) packed into
    # the stationary's M columns: stationary [128, 4] = (whc, whs, wlc, wls).
    # Two moving passes (xh, xl) against the same stationary accumulate the
    # full (xh+xl)*(wh+wl) split across PSUM rows {0,2} (cos) / {1,3} (sin);
    # the host sums even/odd rows.  PE cost: 2 passes, M is free.
    "bf16p": {
        "x": [("xh", _bf16), ("xl", _bf16)],
        "w": [("w4", _bf16, 4)],
        "streams": [(0, 0, 0), (1, 0, 0)],
    },
    # like bf16p but the xl residual is shipped as scaled fp8 (3 B/elem of
    # DMA instead of 4) and multiplies a scaled fp8 copy of w in its own
    # PSUM group. The fp8 stationary carries 4 columns: (wb_c, wb_s) plus a
    # second-order correction pair (wb2_c, wb2_s) encoding the fp8
    # quantization error of wb — M-packing makes the correction free. The
    # host unscales group-1 rows {0,1} by 1/(XL_SCALE*WB_SCALE) and rows
    # {2,3} by 1/(XL_SCALE*WB2_SCALE).
    "bf8p": {
        "x": [("xh", _bf16), ("xl", _f8)],
        "w": [("w4", _bf16, 4), ("wb", _f8, 4)],
        "streams": [(0, 0, 0), (1, 1, 1)],
    },
    # single fp16 stream: x shipped as fp16 (2 B/elem — 50% of the fp32
    # roofline's traffic, 2/3 of bf8p's), stationary fp16 [128, 4] packing
    # (whc, whs, wlc*2^10, wls*2^10) — the w residual columns ride free in M.
    # The host divides PSUM rows {2,3} by 2^10 and sums hi+lo pairs. One
    # moving pass also halves the PE work. Measured max rel err ~1.5e-3.
    "f16p": None,  # dedicated builder (_build_bass_f16)
    "f32r": {
        "x": [("x", _f32r)],
        "w": [("w", _f32r, 2)],
        "streams": [(0, 0, 0)],
    },
    "f16x2": {
        "x": [("xh", _f16), ("xl", _f16)],
        "w": [("wh", _f16, 2)],
        "streams": [(0, 0, 0), (1, 0, 0)],
    },
    "bf16x3": {
        "x": [("xh", _bf16), ("xl", _bf16)],
        "w": [("wh", _bf16, 2), ("wl", _bf16, 2)],
        "streams": [(0, 0, 0), (0, 1, 0), (1, 0, 0)],
    },
}


WL_SCALE = 1024.0  # keeps the fp16 w-residual columns in normal range

# x chunk column ranges (per-core x is [P, RS*B (+ 64 w cols)] fp16; columns
# are rs-major, batch-minor). Full 2-rs chunks stream at peak DMA rate; the
# packed w4 columns ride at the end of chunk 0 so no separate weight DMA
# touches the DMA engines. rs15 arrives as four 256-col quarters (182 ns
# each) so the final matmuls + PSUM drains pipeline as early as possible.
_Q = N_HALF // 2  # 256
W_COLS = RS * 4  # 64 packed stationary columns riding in chunk 0
# (dram_col0, logical_col0, ncols): dram layout is [rs0 rs1 | w4 | rs2..rs15]
F16_CHUNKS = (
    [(0, 0, 2 * B + W_COLS)]
    + [(rs * B + W_COLS, rs * B, 2 * B) for rs in range(2, 14, 2)]
    + [(14 * B + W_COLS, 14 * B, B)]  # rs14
    + [(15 * B + W_COLS + q * _Q, 15 * B + q * _Q, _Q) for q in range(4)]
)


def _build_bass_f16(repeat=1):
    nc = bacc.Bacc("TRN2", debug=False)
    o_d = nc.dram_tensor("o", [4, B], _f32, kind="ExternalOutput").ap()
    x_d = nc.dram_tensor("xh", [P, RS * B + W_COLS], _f16, kind="ExternalInput").ap()

    with tile.TileContext(nc) as tc:
        with (
            tc.tile_pool(name="xp", bufs=len(F16_CHUNKS)) as xp,
            tc.tile_pool(name="wp", bufs=1) as wp,
            tc.tile_pool(name="pp", bufs=1, space="PSUM") as pp,
            tc.tile_pool(name="op", bufs=1) as op,
        ):
            # output staging: rows 0..3 collect the four PSUM quarter drains
            osrc = op.tile([4, B], _f32, name="osrc", tag="osrc")

            # PE warm-up junk matmuls: keep the PE busy during the first
            # chunk's DMA so the p-state ramp (0.65 -> 1.2 -> 2.4 GHz after
            # 3 us busy) completes before the real matmuls start
            junk = wp.tile([P, N_HALF], _f16, name="junk", tag="junk")
            nc.vector.memset(junk[:], 0.0)
            scratch = pp.tile([4, N_HALF], _f32, name="scratch", tag="scratch", bufs=1)
            for _ in range(8):
                nc.tensor.matmul(
                    scratch[:2, :], junk[:, 0:2], junk[:], start=True, stop=True
                )

            for it in range(repeat):
                # one PSUM accumulator (own bank) per 256-col batch quarter:
                # separate banks keep the start=True zeroing (2 KiB region
                # granular) independent, and separate tiles let each
                # quarter's drain copy depend only on its own matmuls
                ps = [
                    pp.tile([4, N_HALF], _f32, name=f"ps{q}_{it}", tag=f"ps{q}")
                    for q in range(4)
                ]
                w_sb = None
                for c, (dcol0, col0, ncols) in enumerate(F16_CHUNKS):
                    x_c = xp.tile([P, ncols], _f16, name=f"x_{it}_{c}", tag="x")
                    nc.sync.dma_start(x_c[:], x_d[:, dcol0 : dcol0 + ncols])
                    if c == 0:
                        w_sb = x_c  # stationary slices live at cols 2*B..
                    xcols = ncols - (W_COLS if c == 0 else 0)
                    off = 0
                    while off < xcols:
                        bcol = (col0 + off) % B  # batch col within the row
                        rs = (col0 + off) // B
                        q = bcol // _Q
                        n = min(_Q, xcols - off)
                        nc.tensor.matmul(
                            ps[q][:, :n],
                            w_sb[:, 2 * B + 4 * rs : 2 * B + 4 * (rs + 1)],
                            x_c[:, off : off + n],
                            start=(rs == 0),
                            stop=(rs == RS - 1),
                        )
                        if rs == RS - 1:
                            # drain each PSUM quarter as soon as it stops,
                            # alternating engines so copies overlap
                            copy = nc.scalar.copy if q % 2 == 0 else (
                                nc.vector.tensor_copy
                            )
                            copy(osrc[:, bcol : bcol + n], ps[q][:, :n])
                        off += n
                nc.sync.dma_start(o_d[:], osrc[:])
    nc.compile()
    return nc


def _build_bass(repeat=1, precision="bf8p"):
    if precision == "f16p":
        return _build_bass_f16(repeat)
    cfg = _CONFIGS[precision]
    x_specs, w_specs, streams = cfg["x"], cfg["w"], cfg["streams"]
    groups = sorted({g for _, _, g in streams})
    grp_m = {g: max(w_specs[wi][2] for _, wi, gg in streams if gg == g) for g in groups}
    m_max = max(grp_m.values())

    nc = bacc.Bacc("TRN2", debug=False)
    # one output tensor: group g occupies columns [g*B, (g+1)*B) — both
    # PSUM groups sit on partitions 0..m-1, so a single SBUF staging tile
    # and a single out-DMA cover all of them
    o_d = nc.dram_tensor(
        "o", [m_max, len(groups) * B], _f32, kind="ExternalOutput"
    ).ap()
    x_ds = [
        nc.dram_tensor(n, [P, RS * B], dt, kind="ExternalInput").ap()
        for n, dt in x_specs
    ]
    w_ds = [
        nc.dram_tensor(n, [P, RS * m], dt, kind="ExternalInput").ap()
        for n, dt, m in w_specs
    ]

    with tile.TileContext(nc) as tc:
        with (
            tc.tile_pool(name="xp", bufs=N_CHUNKS) as xp,
            tc.tile_pool(name="wp", bufs=1) as wp,
            tc.tile_pool(name="pp", bufs=1, space="PSUM") as pp,
            tc.tile_pool(name="op", bufs=2) as op,
        ):
            # weights go via SWDGE (gpsimd) so the x chunk DMAs own the
            # HWDGE ring from t=0
            w_sbs = []
            for i, (n, dt, m) in enumerate(w_specs):
                w_sb = wp.tile([P, RS * m], dt, name=f"w_sb{i}", tag=f"w{i}")
                nc.gpsimd.dma_start(w_sb[:], w_ds[i][:])
                w_sbs.append(w_sb)

            # PE warm-up: junk matmuls on a zeroed tile keep the PE busy
            # during the first chunk's DMA so the HAM clock-gate releases
            # (1.2 -> 2.4 GHz) before the real matmuls start. The final
            # junk matmuls read the w_sb tiles so the first real matmul
            # carries a single sync-wait (the fused LDW+MM pair has few
            # wait slots).
            junk = wp.tile([P, N_HALF], x_specs[0][1], name="junk", tag="junk")
            nc.vector.memset(junk[:], 0.0)
            scratch = pp.tile(
                [max(grp_m.values()), N_HALF],
                _f32,
                name="scratch",
                tag="scratch",
                bufs=1,
            )
            for _ in range(8):
                nc.tensor.matmul(
                    scratch[:2, :], junk[:, 0:2], junk[:], start=True, stop=True
                )
            for i, w_sb in enumerate(w_sbs):
                m = w_specs[i][2]
                nc.tensor.matmul(
                    scratch[:m, 0 : RS * m],
                    w_sb[:, 0:m],
                    w_sb[:],
                    start=True,
                    stop=True,
                )

            # chunk sizes in rs columns; a small final chunk shortens the
            # post-DMA matmul tail
            chunk_rs = [RS_PER_CHUNK] * (RS // RS_PER_CHUNK - 1) + [1] * (
                RS_PER_CHUNK
            )
            assert sum(chunk_rs) == RS

            first_s = {
                g: next(s for s in streams if s[2] == g) for g in groups
            }
            last_s = {
                g: next(s for s in reversed(streams) if s[2] == g)
                for g in groups
            }

            for it in range(repeat):
                ps = {
                    (g, h): pp.tile(
                        [grp_m[g], N_HALF],
                        _f32,
                        name=f"ps{g}_{h}_{it}",
                        tag=f"ps{g}_{h}",
                    )
                    for g in groups
                    for h in range(2)
                }
                rs0 = 0
                for c, crs in enumerate(chunk_rs):
                    x_cs = []
                    for i, (n, dt) in enumerate(x_specs):
                        x_c = xp.tile(
                            [P, crs * B], dt, name=f"x{i}_{it}_{c}", tag=f"x{i}"
                        )
                        nc.sync.dma_start(
                            x_c[:], x_ds[i][:, rs0 * B : (rs0 + crs) * B]
                        )
                        x_cs.append(x_c)
                    for r in range(crs):
                        rs = rs0 + r
                        last_rs = rs == RS - 1
                        # streams stay in xh-first order: on the final column
                        # the xh-dependent matmuls start as soon as xh lands
                        # (xl arrives last), and each bank's copy launches
                        # right after its own final matmul
                        s_order = streams
                        for s in s_order:
                            xi, wi, g = s
                            m = w_specs[wi][2]
                            lhsT = w_sbs[wi][:, m * rs : m * (rs + 1)]
                            # on the final column, finish bank 1 first so its
                            # PSUM->SBUF copy overlaps bank 0's last matmuls
                            h_order = (1, 0) if last_rs else (0, 1)
                            for h in h_order:
                                rhs = x_cs[xi][
                                    :, r * B + h * N_HALF : r * B + (h + 1) * N_HALF
                                ]
                                nc.tensor.matmul(
                                    ps[(g, h)][:],
                                    lhsT,
                                    rhs,
                                    start=(rs == 0 and s == first_s[g]),
                                    stop=(last_rs and s == last_s[g]),
                                )
                    rs0 += crs

                out_sb = op.tile(
                    [m_max, len(groups) * B],
                    _f32,
                    name=f"out_sb_{it}",
                    tag="out_sb",
                )
                for g in groups:
                    m = grp_m[g]
                    nc.vector.tensor_copy(
                        out_sb[:m, g * B + N_HALF : (g + 1) * B], ps[(g, 1)][:]
                    )
                    nc.scalar.copy(
                        out_sb[:m, g * B : g * B + N_HALF], ps[(g, 0)][:]
                    )
                nc.sync.dma_start(o_d[:], out_sb[:])
    nc.compile()
    return nc


def _get_nc(repeat=1, precision="bf8p"):
    key = (repeat, precision)
    if key not in _NC_CACHE:
        _NC_CACHE[key] = _build_bass(repeat, precision)
    return _NC_CACHE[key]


def _fold_weights(freqs):
    """Fold freqs -> effective per-position cos/sin weights [S, 2] (f32)."""
    ang = 2.0 * np.pi * np.asarray(freqs, dtype=np.float64)
    cosv = np.cos(ang)
    msinv = -np.sin(ang)
    c_eff = np.zeros(S, np.float64)
    s_eff = np.zeros(S, np.float64)
    for w in range(NWIN):
        c_eff[w * STEP : w * STEP + NPERSEG] += cosv
        s_eff[w * STEP : w * STEP + NPERSEG] += msinv
    c_eff /= NWIN
    s_eff /= NWIN
    return np.stack([c_eff, s_eff], axis=-1).astype(np.float32)  # [S, 2]


def _pow2_scale(max_abs, target=120.0):
    """Largest power-of-2 scale keeping max_abs*scale <= target.

    ml_dtypes.float8_e4m3 (IEEE, used for mybir float8e4) has max finite
    240 and overflows to inf — stay at half that."""
    if max_abs <= 0 or not np.isfinite(max_abs):
        return 1.0
    return float(2.0 ** np.floor(np.log2(target / max_abs)))


def _run_f16(input, freqs, fc_w, fc_b, trace=False):
    input = np.ascontiguousarray(np.asarray(input, dtype=np.float32))
    eff = _fold_weights(freqs)  # [S, 2] f32

    # device layout x[p, rs*B + b] = shard[b, p*RS + rs]
    x_dev = input.reshape(B, N_CORES, P, RS).transpose(1, 2, 3, 0)
    w_dev = eff.reshape(N_CORES, P, RS, 2)

    in_maps = []
    for k in range(N_CORES):
        xh = x_dev[k].reshape(P, RS * B).astype(np.float16)
        w2 = w_dev[k].astype(np.float64)
        wh = w2.astype(np.float16)
        wl = ((w2 - wh.astype(np.float64)) * WL_SCALE).astype(np.float16)
        w4 = np.concatenate([wh, wl], axis=-1).reshape(P, RS * 4)
        # dram layout [rs0 rs1 | w4 | rs2..rs15]: w4 rides in chunk 0's DMA
        in_maps.append(
            {
                "xh": np.ascontiguousarray(
                    np.concatenate(
                        [xh[:, : 2 * B], w4, xh[:, 2 * B :]], axis=1
                    )
                )
            }
        )

    last_exc = None
    for attempt in range(3):
        try:
            res = run_bass_kernel_spmd(
                _get_nc(1, "f16p"),
                in_maps,
                core_ids=list(range(N_CORES)),
                trace=trace,
            )
            break
        except Exception as e:  # transient NRT/device hiccups: retry
            last_exc = e
            import time as _time

            _time.sleep(2.0)
    else:
        raise last_exc

    fr = np.zeros(B, np.float64)
    fi = np.zeros(B, np.float64)
    for r in res.results:
        o = r["o"].astype(np.float64)  # [4, B]: (hi_c, hi_s, lo_c, lo_s)
        fr += o[0] + o[2] / WL_SCALE
        fi += o[1] + o[3] / WL_SCALE
    psd = fr**2 + fi**2
    out = psd * float(np.asarray(fc_w).reshape(-1)[0]) + float(
        np.asarray(fc_b).reshape(-1)[0]
    )
    return out.astype(np.float32).reshape(B, 1), res


def _run(input, freqs, fc_w, fc_b, trace=False, precision="bf8p"):
    if precision == "f16p":
        return _run_f16(input, freqs, fc_w, fc_b, trace=trace)
    input = np.ascontiguousarray(np.asarray(input, dtype=np.float32))
    eff = _fold_weights(freqs)

    # rearrange to the device layout x[p, rs*B + b] = shard[b, p*RS + rs]
    x_dev = np.ascontiguousarray(
        input.reshape(B, N_CORES, P, RS).transpose(1, 2, 3, 0)
    )  # [N_CORES, P, RS, B]
    w_dev = eff.reshape(N_CORES, P, RS * 2)

    # adaptive (host-side only) fp8 scales: the device multiplies scaled
    # values, the host divides the partials back down
    scales = {}
    if precision == "bf8p":
        import ml_dtypes

        f8_np = mybir.dt.np(_f8)
        xl_all = input - input.astype(ml_dtypes.bfloat16).astype(np.float32)
        scales["xl"] = _pow2_scale(np.abs(xl_all).max())
        scales["wb"] = _pow2_scale(np.abs(eff).max())
        werr_all = eff - (eff * scales["wb"]).astype(f8_np).astype(
            np.float32
        ) / scales["wb"]
        scales["wb2"] = _pow2_scale(np.abs(werr_all).max())
        del xl_all, werr_all

    in_maps = []
    for k in range(N_CORES):
        x_host = x_dev[k].reshape(P, RS * B)
        w_host = w_dev[k]
        if precision in ("bf16p", "bf8p"):
            import ml_dtypes

            xh = x_host.astype(ml_dtypes.bfloat16)
            xl_f32 = x_host - xh.astype(np.float32)
            w2 = w_host.reshape(P, RS, 2)
            wh = w2.astype(ml_dtypes.bfloat16)
            wl = (w2 - wh.astype(np.float32)).astype(ml_dtypes.bfloat16)
            w4 = np.concatenate([wh, wl], axis=-1).reshape(P, RS * 4)
            m = {"xh": xh, "w4": np.ascontiguousarray(w4)}
            if precision == "bf16p":
                m["xl"] = np.ascontiguousarray(xl_f32.astype(ml_dtypes.bfloat16))
            else:
                f8 = mybir.dt.np(_f8)
                xl_s, wb_s, wb2_s = scales["xl"], scales["wb"], scales["wb2"]
                m["xl"] = np.ascontiguousarray((xl_f32 * xl_s).astype(f8))
                wb = (w2 * wb_s).astype(f8)
                werr = w2 - wb.astype(np.float32) / wb_s
                wb2 = (werr * wb2_s).astype(f8)
                m["wb"] = np.ascontiguousarray(
                    np.concatenate([wb, wb2], axis=-1).reshape(P, RS * 4)
                )
            in_maps.append(m)
        elif precision == "f32r":
            in_maps.append({"x": x_host, "w": np.ascontiguousarray(w_host)})
        elif precision == "f16x2":
            xh = x_host.astype(np.float16)
            xl = (x_host - xh.astype(np.float32)).astype(np.float16)
            wh = np.ascontiguousarray(w_host).astype(np.float16)
            in_maps.append({"xh": xh, "xl": np.ascontiguousarray(xl), "wh": wh})
        else:
            import ml_dtypes

            xh = x_host.astype(ml_dtypes.bfloat16)
            xl = (x_host - xh.astype(np.float32)).astype(ml_dtypes.bfloat16)
            wh = w_host.astype(ml_dtypes.bfloat16)
            wl = (w_host - wh.astype(np.float32)).astype(ml_dtypes.bfloat16)
            in_maps.append(
                {
                    "xh": xh,
                    "xl": np.ascontiguousarray(xl),
                    "wh": np.ascontiguousarray(wh),
                    "wl": np.ascontiguousarray(wl),
                }
            )

    last_exc = None
    for attempt in range(3):
        try:
            res = run_bass_kernel_spmd(
                _get_nc(1, precision),
                in_maps,
                core_ids=list(range(N_CORES)),
                trace=trace,
            )
            break
        except Exception as e:  # transient NRT/device hiccups: retry
            last_exc = e
            import time as _time

            _time.sleep(2.0)
    else:
        raise last_exc

    fr = np.zeros(B, np.float64)
    fi = np.zeros(B, np.float64)
    for r in res.results:
        o = r["o"]
        g0 = o[:, 0:B]
        fr += g0[0::2].sum(axis=0, dtype=np.float64)
        fi += g0[1::2].sum(axis=0, dtype=np.float64)
        if o.shape[1] > B:  # fp8 residual group (bf8p)
            g1 = o[:, B : 2 * B]
            s1 = scales["xl"] * scales["wb"]
            s2 = scales["xl"] * scales["wb2"]
            fr += g1[0].astype(np.float64) / s1
            fi += g1[1].astype(np.float64) / s1
            fr += g1[2].astype(np.float64) / s2
            fi += g1[3].astype(np.float64) / s2
    psd = fr**2 + fi**2
    out = psd * float(np.asarray(fc_w).reshape(-1)[0]) + float(
        np.asarray(fc_b).reshape(-1)[0]
    )
    return out.astype(np.float32).reshape(B, 1), res


def kernel(input, freqs, fc_w, fc_b):
    out, _ = _run(input, freqs, fc_w, fc_b, trace=False, precision="f16p")
    return out

